# revision 7
# baseline (speedup 1.0000x reference)
"""CstLoss on Trainium2 — self-contained Bass/Tile SPMD kernel (8 NeuronCores).

Reference math (per [N=64, C=17, H=128, W=128] f32 pair output/target):
  h/w marginal means of each map -> softmax over the 128-axis -> l2
  normalize -> sim_pos = mean of matched-channel cosines, sim = sum of
  mean-over-batch all-pairs cosines, loss = -log(sim_pos/sim)/C/N.

Key algebra:
  * softmax denominator AND the max-shift cancel under l2 normalization
    (exp args are S/128 with |S| <= ~128, so exp(S/128) is always in
    [e^-1, e^1.1] -- no overflow), so each projection only needs
    y = exp(S/128); q = y/||y||_2 with S the raw row/col sums.
  * sum_ij dot(qo_i, qt_j) = dot(sum_i qo_i, sum_j qt_j), so the CxC pair
    matrix is never materialized: per n we only need channel sums U, V.

Sharding: data-parallel over batch, 8 entries (136 maps) per core. The host
pre-transposes each core's slice to [h=128, maps=136, w=128] so the kernel
loads with h on partitions (contiguous 8.7KB DMA lines per partition/group).

Per-core kernel:
  h-proj: DVE segmented reduce over w -> SH [128h, 136m].
  w-proj: per tensor, 34 float32r matmuls with one-hot-column weights
    (sliding [128,34] windows of a zeros|ones-col|zeros const) accumulate
    dense rows into one PSUM tile [34, 512] = [34, 4 maps, 128 w]; row k
    holds the w-sums of maps 4k..4k+3. (PE reduces over partitions at
    1 col/cycle in f32r -- this replaces the baseline's 256 serial PE
    transposes, which were the bottleneck.)
  Normalization in both layouts via Exp/Square/Rsqrt + small matmuls; the
  host reduces per-core outputs (matched cosines + rnorm-weighted channel
  sums) to the scalar loss.
"""

import contextlib
import ctypes
import shutil
import sys
import types
from contextlib import ExitStack

import numpy as np

import concourse.bacc as bacc
import concourse.tile as tile
from concourse import mybir
from concourse.bass_utils import run_bass_kernel_spmd

F32 = mybir.dt.float32
F32R = mybir.dt.float32r
AX = mybir.AxisListType
ACT = mybir.ActivationFunctionType

N, C, H, W = 64, 17, 128, 128
NCORES = 8
NLOC = N // NCORES           # 8 batch entries per core
MAPS = NLOC * C              # 136 maps per tensor per core
ROWS = MAPS // 4             # 34 w-proj accumulator rows (4 maps each)
GRP = 17                     # maps per DMA group
NG = MAPS // GRP             # 8 groups per tensor


def _install_ntff_hook():
    """Provide antenv.axon_hooks if the image lacks it (needed only when
    run_bass_kernel_spmd is called with trace=True; harmless otherwise)."""
    if "antenv.axon_hooks" in sys.modules:
        return
    so_path = "/opt/axon/libaxon_pjrt.so"
    hook = None
    try:
        lib = ctypes.CDLL(so_path)
        if hasattr(lib, "axon_start_nrt_profile"):
            lib.axon_start_nrt_profile.argtypes = [
                ctypes.POINTER(ctypes.c_int64),
                ctypes.c_size_t,
            ]
            lib.axon_start_nrt_profile.restype = ctypes.c_int64
            lib.axon_stop_nrt_profile.argtypes = [ctypes.c_char_p]
            lib.axon_stop_nrt_profile.restype = ctypes.c_int64

            @contextlib.contextmanager
            def _hook(output_dir, device_ids):
                import jax

                jax.devices()
                if device_ids:
                    ids = (ctypes.c_int64 * len(device_ids))(*device_ids)
                    rc = lib.axon_start_nrt_profile(ids, len(device_ids))
                else:
                    rc = lib.axon_start_nrt_profile(None, 0)
                if rc != 0:
                    raise RuntimeError(f"axon_start_nrt_profile rc={rc}")
                try:
                    yield
                finally:
                    n = lib.axon_stop_nrt_profile(str(output_dir).encode())
                    print(f"profile: {n} file(s) in {output_dir}", file=sys.stderr)

            hook = _hook
    except OSError:
        pass
    mod = types.ModuleType("antenv.axon_hooks")
    mod.get_axon_ntff_profile_hook = lambda: hook
    mod.set_axon_ntff_profile_hook = lambda h: None
    sys.modules["antenv.axon_hooks"] = mod


_install_ntff_hook()


def _body(tc, o_d, t_d, zoz_d, ones1_d, onesh_d, mask_d,
          ph_d, pw_d, uh_d, vh_d, uw_d, vw_d):
    nc = tc.nc
    with ExitStack() as ctx:
        constp = ctx.enter_context(tc.tile_pool(name="constp", bufs=1))
        datap = ctx.enter_context(tc.tile_pool(name="datap", bufs=1))
        workp = ctx.enter_context(tc.tile_pool(name="workp", bufs=1))
        outp = ctx.enter_context(tc.tile_pool(name="outp", bufs=1))
        # PSUM: distinct tiles only, no slot rotation (slot reuse with
        # concurrent PE traffic wedges the device: NRT status 101).
        psump = ctx.enter_context(tc.tile_pool(name="psump", bufs=1, space="PSUM"))

        # consts on the scalar HWDGE ring: tiny, done by ~3us, and the ACT
        # engine must stay free of big load-DMAs (a HWDGE dma_start occupies
        # the issuing engine for the whole transfer).
        zoz = constp.tile([128, 2 * ROWS - 1], F32R)
        nc.scalar.dma_start(zoz[:], zoz_d)
        ones1 = constp.tile([1, 128], F32)
        nc.scalar.dma_start(ones1[:], ones1_d)
        onesh = constp.tile([128, 1], F32)
        nc.scalar.dma_start(onesh[:], onesh_d)
        mask = constp.tile([ROWS, 32], F32)
        nc.scalar.dma_start(mask[:], mask_d)

        # data + w-proj weights live as float32r (same bits as f32) so the
        # fp32r matmuls' producers satisfy the BIR checkMatmultFP32r rule;
        # non-matmul consumers bitcast back to f32.
        Xo = datap.tile([128, MAPS * W], F32R)
        Xt = datap.tile([128, MAPS * W], F32R)

        # ---- loads: o first on both queues (sync HWDGE + gpsimd SWDGE),
        # then t; o completes ~halfway so its norm chain overlaps t's load ----
        for X, x_d in ((Xo, o_d), (Xt, t_d)):
            for g in range(NG):
                e = nc.sync if (g < NG // 2) else nc.gpsimd
                e.dma_start(
                    X[:, g * GRP * W:(g + 1) * GRP * W],
                    x_d[:, g * GRP:(g + 1) * GRP, :],
                )

        # ---- h-proj: DVE segmented reduce over w, per group as it lands ----
        SHo = workp.tile([128, MAPS], F32)
        SHt = workp.tile([128, MAPS], F32)
        for gi in range(2 * NG):
            # issue order matches arrival: (o0,o4),(o1,o5),...,(t0,t4),...
            X, SH = (Xo, SHo) if gi < NG else (Xt, SHt)
            g = (gi % NG) // 2 + (NG // 2) * (gi % 2)
            v = X[:, g * GRP * W:(g + 1) * GRP * W].bitcast(F32).rearrange(
                "p (m w) -> p m w", w=W
            )
            nc.vector.reduce_sum(SH[:, g * GRP:(g + 1) * GRP], v, axis=AX.X)

        # ---- w-proj: one-hot-column f32r matmuls, dense rows in PSUM ----
        Wacc_o = psump.tile([ROWS, 512], F32)
        Wacc_t = psump.tile([ROWS, 512], F32)
        for X, Wacc in ((Xo, Wacc_o), (Xt, Wacc_t)):
            for k in range(ROWS):
                nc.tensor.matmul(
                    Wacc[:],
                    zoz[:, ROWS - 1 - k:2 * ROWS - 1 - k],
                    X[:, k * 512:(k + 1) * 512],
                    start=(k == 0),
                    stop=(k == ROWS - 1),
                )

        # ---- w-side normalization & stats ([34, 4 maps, 128 w]) ----
        YWo = workp.tile([ROWS, 512], F32)
        YWt = workp.tile([ROWS, 512], F32)
        Y2W = workp.tile([ROWS, 512], F32)
        ssqW = workp.tile([ROWS, 8], F32)
        sqW = workp.tile([ROWS, 8], F32)
        rnW = workp.tile([ROWS, 8], F32)
        for i, (Wacc, YW) in enumerate(((Wacc_o, YWo), (Wacc_t, YWt))):
            nc.scalar.activation(YW[:], Wacc[:], ACT.Exp, scale=1.0 / W)
            nc.scalar.activation(Y2W[:], YW[:], ACT.Square)
            nc.vector.reduce_sum(
                ssqW[:, 4 * i:4 * i + 4],
                Y2W.rearrange("p (m w) -> p m w", w=W),
                axis=AX.X,
            )
        nc.scalar.activation(sqW[:], ssqW[:], ACT.Sqrt)
        nc.vector.reciprocal(rnW[:], sqW[:])

        DW = workp.tile([ROWS, 512], F32)
        nc.vector.tensor_mul(DW[:], YWo[:], YWt[:])
        posW = outp.tile([ROWS, 4], F32)
        nc.vector.reduce_sum(
            posW[:], DW.rearrange("p (m w) -> p m w", w=W), axis=AX.X
        )
        nc.vector.tensor_mul(posW[:], posW[:], rnW[:, 0:4])
        nc.vector.tensor_mul(posW[:], posW[:], rnW[:, 4:8])
        nc.sync.dma_start(pw_d, posW[:])

        # ---- w-side channel sums: rnorm-weighted matmuls over rows ----
        UVo = psump.tile([NLOC, W], F32)
        UVt = psump.tile([NLOC, W], F32)
        for i, (YW, UV) in enumerate(((YWo, UVo), (YWt, UVt))):
            for g in range(4):
                Wg = workp.tile(
                    [ROWS, NLOC], F32, tag="wg", name=f"wg{i}_{g}"
                )
                nc.vector.tensor_scalar_mul(
                    Wg[:], mask[:, NLOC * g:NLOC * (g + 1)],
                    rnW[:, 4 * i + g:4 * i + g + 1],
                )
                nc.tensor.matmul(
                    UV[:], Wg[:], YW[:, g * W:(g + 1) * W],
                    start=(g == 0), stop=(g == 3),
                )
        uvW = outp.tile([NLOC, 2 * W], F32)
        nc.scalar.copy(uvW[:, 0:W], UVo[:])
        nc.scalar.copy(uvW[:, W:2 * W], UVt[:])
        nc.scalar.dma_start(uw_d, uvW[:, 0:W])
        nc.scalar.dma_start(vw_d, uvW[:, W:2 * W])

        # ---- h-side normalization ([128 h, 136 m], per-map = per-column) ----
        YHo = workp.tile([128, MAPS], F32)
        YHt = workp.tile([128, MAPS], F32)
        Y2H = workp.tile([128, MAPS], F32)
        DH = workp.tile([128, MAPS], F32)
        rowp = psump.tile([1, 512], F32)
        for i, (SH, YH) in enumerate(((SHo, YHo), (SHt, YHt))):
            nc.scalar.activation(YH[:], SH[:], ACT.Exp, scale=1.0 / W)
            nc.scalar.activation(Y2H[:], YH[:], ACT.Square)
            nc.tensor.matmul(
                rowp[0:1, i * MAPS:(i + 1) * MAPS], onesh[:], Y2H[:],
                skip_group_check=True,
            )
        nc.vector.tensor_mul(DH[:], YHo[:], YHt[:])
        nc.tensor.matmul(
            rowp[0:1, 2 * MAPS:3 * MAPS], onesh[:], DH[:],
            skip_group_check=True,
        )
        sqrow = workp.tile([1, 2 * MAPS], F32)
        nc.scalar.activation(sqrow[:], rowp[0:1, 0:2 * MAPS], ACT.Sqrt)
        rnrow = workp.tile([1, 2 * MAPS], F32)
        nc.vector.reciprocal(rnrow[:], sqrow[:])
        posH = outp.tile([1, MAPS], F32)
        nc.vector.tensor_mul(posH[:], rnrow[:, 0:MAPS], rowp[0:1, 2 * MAPS:3 * MAPS])
        nc.vector.tensor_mul(posH[:], posH[:], rnrow[:, MAPS:2 * MAPS])
        nc.sync.dma_start(ph_d, posH[:])

        # ---- h-side channel sums: q = y * bcast(rnorm); reduce c-segments ----
        BCo = psump.tile([128, MAPS], F32)
        BCt = psump.tile([128, MAPS], F32)
        uvH = outp.tile([128, 2 * NLOC], F32)
        QH = workp.tile([128, MAPS], F32)
        for i, (YH, BC) in enumerate(((YHo, BCo), (YHt, BCt))):
            nc.tensor.matmul(
                BC[:], ones1[:], rnrow[0:1, i * MAPS:(i + 1) * MAPS]
            )
            nc.vector.tensor_mul(QH[:], YH[:], BC[:])
            nc.vector.reduce_sum(
                uvH[:, i * NLOC:(i + 1) * NLOC],
                QH.rearrange("p (n c) -> p n c", c=C),
                axis=AX.X,
            )
        nc.gpsimd.dma_start(uh_d, uvH[:, 0:NLOC])
        nc.gpsimd.dma_start(vh_d, uvH[:, NLOC:2 * NLOC])


def _build_nc():
    nc = bacc.Bacc("TRN2", target_bir_lowering=False, debug=False)
    o_d = nc.dram_tensor("o", [128, MAPS, W], F32R, kind="ExternalInput").ap()
    t_d = nc.dram_tensor("t", [128, MAPS, W], F32R, kind="ExternalInput").ap()
    zoz_d = nc.dram_tensor("zoz", [128, 2 * ROWS - 1], F32R, kind="ExternalInput").ap()
    ones1_d = nc.dram_tensor("ones1", [1, 128], F32, kind="ExternalInput").ap()
    onesh_d = nc.dram_tensor("onesh", [128, 1], F32, kind="ExternalInput").ap()
    mask_d = nc.dram_tensor("mask", [ROWS, 32], F32, kind="ExternalInput").ap()
    ph_d = nc.dram_tensor("ph", [1, MAPS], F32, kind="ExternalOutput").ap()
    pw_d = nc.dram_tensor("pw", [ROWS, 4], F32, kind="ExternalOutput").ap()
    uh_d = nc.dram_tensor("uh", [128, NLOC], F32, kind="ExternalOutput").ap()
    vh_d = nc.dram_tensor("vh", [128, NLOC], F32, kind="ExternalOutput").ap()
    uw_d = nc.dram_tensor("uw", [NLOC, W], F32, kind="ExternalOutput").ap()
    vw_d = nc.dram_tensor("vw", [NLOC, W], F32, kind="ExternalOutput").ap()
    with tile.TileContext(nc) as tc:
        _body(tc, o_d, t_d, zoz_d, ones1_d, onesh_d, mask_d,
              ph_d, pw_d, uh_d, vh_d, uw_d, vw_d)
    nc.compile()
    return nc


_NC = None


def _get_nc():
    global _NC
    if _NC is None:
        _NC = _build_nc()
    return _NC


_ZOZ = np.zeros((128, 2 * ROWS - 1), np.float32)
_ZOZ[:, ROWS - 1] = 1.0
_ONES1 = np.ones((1, 128), np.float32)
_ONESH = np.ones((128, 1), np.float32)
_MASK = np.zeros((ROWS, 32), np.float32)
for _r in range(ROWS):
    for _g in range(4):
        _MASK[_r, NLOC * _g + (4 * _r + _g) // C] = 1.0


def _make_in_maps(output, target):
    in_maps = []
    for i in range(NCORES):
        o = np.ascontiguousarray(
            output[i * NLOC:(i + 1) * NLOC].reshape(MAPS, H, W).transpose(1, 0, 2)
        )
        t = np.ascontiguousarray(
            target[i * NLOC:(i + 1) * NLOC].reshape(MAPS, H, W).transpose(1, 0, 2)
        )
        in_maps.append(
            {"o": o, "t": t, "zoz": _ZOZ, "ones1": _ONES1,
             "onesh": _ONESH, "mask": _MASK}
        )
    return in_maps


def _finish(results):
    A = 0.0
    B = 0.0
    for r in results:
        A += float(r["ph"].astype(np.float64).sum())
        A += float(r["pw"].astype(np.float64).sum())
        B += float((r["uh"].astype(np.float64) * r["vh"].astype(np.float64)).sum())
        B += float((r["uw"].astype(np.float64) * r["vw"].astype(np.float64)).sum())
    # sim_pos = 0.5*A/(N*C); sim = 0.5*B/N; loss = -log(sim_pos/sim)/(C*N)
    loss = -np.log(A / (C * B)) / (C * N)
    return np.float32(loss)


def kernel(output, target):
    output = np.asarray(output, dtype=np.float32)
    target = np.asarray(target, dtype=np.float32)
    nc = _get_nc()
    res = run_bass_kernel_spmd(nc, _make_in_maps(output, target), list(range(NCORES)))
    return _finish(res.results)


def profile(output, target):
    """Run once with NTFF tracing; returns max per-core HW exec time in ns."""
    output = np.asarray(output, dtype=np.float32)
    target = np.asarray(target, dtype=np.float32)
    nc = _get_nc()
    tmpdir = "/tmp/bass_prof_latest"
    shutil.rmtree(tmpdir, ignore_errors=True)
    import os

    os.makedirs(tmpdir, exist_ok=True)
    res = run_bass_kernel_spmd(
        nc, _make_in_maps(output, target), list(range(NCORES)), trace=True,
        tmpdir=tmpdir,
    )
    return res.exec_time_ns


# revision 8
# speedup vs baseline: 1.0937x; 1.0937x over previous
"""CstLoss on Trainium2 — self-contained Bass/Tile SPMD kernel (8 NeuronCores).

Reference math (per [N=64, C=17, H=128, W=128] f32 pair output/target):
  h/w marginal means of each map -> softmax over the 128-axis -> l2
  normalize -> sim_pos = mean of matched-channel cosines, sim = sum of
  mean-over-batch all-pairs cosines, loss = -log(sim_pos/sim)/C/N.

Key algebra:
  * softmax denominator AND the max-shift cancel under l2 normalization
    (exp args are S/128 with bounded S, so exp never overflows), so each
    projection only needs y = exp(S/128); q = y/||y||_2, S = raw sums.
  * sum_ij dot(qo_i, qt_j) = dot(sum_i qo_i, sum_j qt_j), so the CxC pair
    matrix is never materialized: per n we only need channel sums U, V.
  * the scalar loss only needs A = sum of matched cosines and
    B = sum_n U_n.V_n; both are computed on-chip, one [1,4] output/core.

Sharding: data-parallel over batch, 8 entries (136 maps) per core. The host
pre-transposes each core's slice to [h=128, maps=136, w=128] so the kernel
loads with h on partitions (17.4KB contiguous DMA lines per partition).

Per-core kernel:
  h-proj: DVE segmented reduce over w -> SH [128h, 136m].
  w-proj: 34 float32r matmul pairs (o_k, t_k share one weight load) with
    one-hot-column weights (sliding [128,34] windows of a zeros|ones|zeros
    const) accumulate dense rows into PSUM [34, 512] = [34, 4m, 128w].
    PE reduces over partitions at ~1 col/cycle in f32r — this replaces the
    baseline's 256 serial PE transposes, which were the bottleneck.
  Normalization via Exp/Square/Sqrt + reciprocal + small matmuls; channel
  sums U/V via rnorm-weighted matmuls (w-side) and a PE row-broadcast +
  segmented reduce (h-side); A/B reduced on-chip.
"""

import contextlib
import ctypes
import os
import shutil
import sys
import types
from contextlib import ExitStack

import numpy as np

import concourse.bacc as bacc
import concourse.tile as tile
from concourse import mybir
from concourse.bass_utils import run_bass_kernel_spmd

F32 = mybir.dt.float32
F32R = mybir.dt.float32r
AX = mybir.AxisListType
ACT = mybir.ActivationFunctionType

N, C, H, W = 64, 17, 128, 128
NCORES = 8
NLOC = N // NCORES           # 8 batch entries per core
MAPS = NLOC * C              # 136 maps per tensor per core
ROWS = MAPS // 4             # 34 w-proj accumulator rows (4 maps each)

# DMA map-groups per (tensor, queue): o in big halves-of-halves, t with a
# small final group so the last-arriving bytes have a short reduce tail.
SYNC_O = [(0, 34), (34, 68)]
GP_O = [(68, 102), (102, 136)]
SYNC_T = [(0, 34), (34, 60), (60, 68)]
GP_T = [(68, 102), (102, 128), (128, 136)]
# DVE reduce issue order ~ arrival order (sync and gpsimd drain in parallel)
REDUCE_ORDER = (
    [("o", g) for pair in zip(SYNC_O, GP_O) for g in pair]
    + [("t", g) for pair in zip(SYNC_T, GP_T) for g in pair]
)


def _install_ntff_hook():
    """Provide antenv.axon_hooks if the image lacks it (needed only when
    run_bass_kernel_spmd is called with trace=True; harmless otherwise)."""
    if "antenv.axon_hooks" in sys.modules:
        return
    so_path = "/opt/axon/libaxon_pjrt.so"
    hook = None
    try:
        lib = ctypes.CDLL(so_path)
        if hasattr(lib, "axon_start_nrt_profile"):
            lib.axon_start_nrt_profile.argtypes = [
                ctypes.POINTER(ctypes.c_int64),
                ctypes.c_size_t,
            ]
            lib.axon_start_nrt_profile.restype = ctypes.c_int64
            lib.axon_stop_nrt_profile.argtypes = [ctypes.c_char_p]
            lib.axon_stop_nrt_profile.restype = ctypes.c_int64

            @contextlib.contextmanager
            def _hook(output_dir, device_ids):
                import jax

                jax.devices()
                if device_ids:
                    ids = (ctypes.c_int64 * len(device_ids))(*device_ids)
                    rc = lib.axon_start_nrt_profile(ids, len(device_ids))
                else:
                    rc = lib.axon_start_nrt_profile(None, 0)
                if rc != 0:
                    raise RuntimeError(f"axon_start_nrt_profile rc={rc}")
                try:
                    yield
                finally:
                    n = lib.axon_stop_nrt_profile(str(output_dir).encode())
                    print(f"profile: {n} file(s) in {output_dir}", file=sys.stderr)

            hook = _hook
    except OSError:
        pass
    mod = types.ModuleType("antenv.axon_hooks")
    mod.get_axon_ntff_profile_hook = lambda: hook
    mod.set_axon_ntff_profile_hook = lambda h: None
    sys.modules["antenv.axon_hooks"] = mod


_install_ntff_hook()


def _body(tc, o_d, t_d, zoz_d, mask_d, ab_d):
    nc = tc.nc
    with ExitStack() as ctx:
        constp = ctx.enter_context(tc.tile_pool(name="constp", bufs=1))
        datap = ctx.enter_context(tc.tile_pool(name="datap", bufs=1))
        workp = ctx.enter_context(tc.tile_pool(name="workp", bufs=1))
        outp = ctx.enter_context(tc.tile_pool(name="outp", bufs=1))
        # PSUM: 8 distinct tiles = 8 banks, no slot rotation (slot reuse
        # with concurrent PE traffic wedges the device: NRT status 101).
        psump = ctx.enter_context(tc.tile_pool(name="psump", bufs=1, space="PSUM"))

        # small consts on the scalar HWDGE ring (ACT stays free of big DMAs)
        zoz = constp.tile([128, 2 * ROWS - 1], F32R)
        nc.scalar.dma_start(zoz[:], zoz_d)
        mask = constp.tile([ROWS, 32], F32)
        nc.scalar.dma_start(mask[:], mask_d)
        ones1 = constp.tile([1, 128], F32)
        nc.vector.memset(ones1[:], 1.0)
        onesh = constp.tile([128, 1], F32)
        nc.vector.memset(onesh[:], 1.0)

        # data as float32r (same bits as f32) so the fp32r matmuls'
        # producers satisfy BIR checkMatmultFP32r; non-matmul readers
        # bitcast back to f32.
        Xo = datap.tile([128, MAPS * W], F32R)
        Xt = datap.tile([128, MAPS * W], F32R)

        # ---- loads: o first on both queues (sync HWDGE + gpsimd SWDGE) ----
        for X, x_d, groups in (
            (Xo, o_d, SYNC_O), (Xt, t_d, SYNC_T)
        ):
            for a, b in groups:
                nc.sync.dma_start(X[:, a * W:b * W], x_d[:, a:b, :])
        for X, x_d, groups in (
            (Xo, o_d, GP_O), (Xt, t_d, GP_T)
        ):
            for a, b in groups:
                nc.gpsimd.dma_start(X[:, a * W:b * W], x_d[:, a:b, :])

        # ---- h-proj: DVE segmented reduce over w, per group as it lands ----
        SHo = workp.tile([128, MAPS], F32)
        SHt = workp.tile([128, MAPS], F32)
        for which, (a, b) in REDUCE_ORDER:
            X, SH = (Xo, SHo) if which == "o" else (Xt, SHt)
            v = X[:, a * W:b * W].bitcast(F32).rearrange(
                "p (m w) -> p m w", w=W
            )
            nc.vector.reduce_sum(SH[:, a:b], v, axis=AX.X)

        # ---- w-proj: paired one-hot-column f32r matmuls (shared weight) ----
        Wacc_o = psump.tile([ROWS, 512], F32)
        Wacc_t = psump.tile([ROWS, 512], F32)
        for k in range(ROWS):
            wgt = zoz[:, ROWS - 1 - k:2 * ROWS - 1 - k]
            for X, Wacc in ((Xo, Wacc_o), (Xt, Wacc_t)):
                nc.tensor.matmul(
                    Wacc[:], wgt, X[:, k * 512:(k + 1) * 512],
                    start=(k == 0), stop=(k == ROWS - 1),
                    skip_group_check=True,
                )

        # ---- h-side, o half (runs while t still streams) ----
        YHo = workp.tile([128, MAPS], F32)
        Y2Ho = workp.tile([128, MAPS], F32)
        rowp = psump.tile([1, 512], F32)
        nc.scalar.activation(YHo[:], SHo[:], ACT.Exp, scale=1.0 / W)
        nc.scalar.activation(Y2Ho[:], YHo[:], ACT.Square)
        nc.tensor.matmul(
            rowp[0:1, 0:MAPS], onesh[:], Y2Ho[:], skip_group_check=True
        )
        sqrowO = workp.tile([1, MAPS], F32)
        nc.scalar.activation(sqrowO[:], rowp[0:1, 0:MAPS], ACT.Sqrt)
        rnrowO = workp.tile([1, MAPS], F32)
        nc.vector.reciprocal(rnrowO[:], sqrowO[:])
        BCo = psump.tile([128, MAPS], F32)
        nc.tensor.matmul(BCo[:], ones1[:], rnrowO[:])
        QH = workp.tile([128, MAPS], F32)
        nc.vector.tensor_mul(QH[:], YHo[:], BCo[:])
        uh = workp.tile([128, NLOC], F32)
        nc.vector.reduce_sum(
            uh[:], QH.rearrange("p (n c) -> p n c", c=C), axis=AX.X
        )

        # ---- h-side, t half + cross terms (tail) ----
        YHt = workp.tile([128, MAPS], F32)
        HT2 = workp.tile([128, 2 * MAPS], F32)   # [Y2Ht | DH]
        nc.scalar.activation(YHt[:], SHt[:], ACT.Exp, scale=1.0 / W)
        nc.scalar.activation(HT2[:, 0:MAPS], YHt[:], ACT.Square)
        nc.vector.tensor_mul(HT2[:, MAPS:2 * MAPS], YHo[:], YHt[:])
        nc.tensor.matmul(
            rowp[0:1, MAPS:3 * MAPS], onesh[:], HT2[:], skip_group_check=True
        )
        sqrowT = workp.tile([1, MAPS], F32)
        nc.scalar.activation(sqrowT[:], rowp[0:1, MAPS:2 * MAPS], ACT.Sqrt)
        rnrowT = workp.tile([1, MAPS], F32)
        nc.vector.reciprocal(rnrowT[:], sqrowT[:])
        posH = workp.tile([1, MAPS], F32)
        nc.vector.tensor_mul(posH[:], rnrowO[:], rowp[0:1, 2 * MAPS:3 * MAPS])
        nc.vector.tensor_mul(posH[:], posH[:], rnrowT[:])
        ABs = outp.tile([1, 4], F32)
        nc.vector.reduce_sum(
            ABs[0:1, 0:1], posH.rearrange("p (x m) -> p x m", x=1), axis=AX.X
        )
        BCt = psump.tile([128, MAPS], F32)
        nc.tensor.matmul(BCt[:], ones1[:], rnrowT[:])
        QH2 = workp.tile([128, MAPS], F32)
        nc.vector.tensor_mul(QH2[:], YHt[:], BCt[:])
        vh = workp.tile([128, NLOC], F32)
        nc.vector.reduce_sum(
            vh[:], QH2.rearrange("p (n c) -> p n c", c=C), axis=AX.X
        )
        ABp = psump.tile([1, 16], F32)
        uhvh = workp.tile([128, NLOC], F32)
        nc.vector.tensor_mul(uhvh[:], uh[:], vh[:])
        uhvr = workp.tile([128, 1], F32)
        nc.vector.reduce_sum(
            uhvr[:], uhvh.rearrange("p (x n) -> p x n", x=1), axis=AX.X
        )
        nc.tensor.matmul(
            ABp[0:1, 2:3], onesh[:], uhvr[:], skip_group_check=True
        )

        # ---- w-side normalization & stats ([34, 4 maps, 128 w]) ----
        YWo = workp.tile([ROWS, 512], F32)
        YWt = workp.tile([ROWS, 512], F32)
        Y2W = workp.tile([ROWS, 512], F32)
        ssqW = workp.tile([ROWS, 8], F32)
        sqW = workp.tile([ROWS, 8], F32)
        rnW = workp.tile([ROWS, 8], F32)
        for i, (Wacc, YW) in enumerate(((Wacc_o, YWo), (Wacc_t, YWt))):
            nc.scalar.activation(YW[:], Wacc[:], ACT.Exp, scale=1.0 / W)
            nc.scalar.activation(Y2W[:], YW[:], ACT.Square)
            nc.vector.reduce_sum(
                ssqW[:, 4 * i:4 * i + 4],
                Y2W.rearrange("p (m w) -> p m w", w=W),
                axis=AX.X,
            )
        nc.scalar.activation(sqW[:], ssqW[:], ACT.Sqrt)
        nc.vector.reciprocal(rnW[:], sqW[:])

        DW = workp.tile([ROWS, 512], F32)
        nc.vector.tensor_mul(DW[:], YWo[:], YWt[:])
        posW = workp.tile([ROWS, 4], F32)
        nc.vector.reduce_sum(
            posW[:], DW.rearrange("p (m w) -> p m w", w=W), axis=AX.X
        )
        nc.vector.tensor_mul(posW[:], posW[:], rnW[:, 0:4])
        nc.vector.tensor_mul(posW[:], posW[:], rnW[:, 4:8])
        pwr = workp.tile([ROWS, 1], F32)
        nc.vector.reduce_sum(
            pwr[:], posW.rearrange("p (x m) -> p x m", x=1), axis=AX.X
        )
        nc.tensor.matmul(
            ABp[0:1, 0:1], onesh[0:ROWS, :], pwr[:], skip_group_check=True
        )

        # ---- w-side channel sums: rnorm-weighted matmuls over rows ----
        UVo = psump.tile([NLOC, W], F32)
        UVt = psump.tile([NLOC, W], F32)
        for i, (YW, UV) in enumerate(((YWo, UVo), (YWt, UVt))):
            for g in range(4):
                Wg = workp.tile(
                    [ROWS, NLOC], F32, tag=f"wg{i}{g}", name=f"wg{i}_{g}"
                )
                nc.vector.tensor_scalar_mul(
                    Wg[:], mask[:, NLOC * g:NLOC * (g + 1)],
                    rnW[:, 4 * i + g:4 * i + g + 1],
                )
                nc.tensor.matmul(
                    UV[:], Wg[:], YW[:, g * W:(g + 1) * W],
                    start=(g == 0), stop=(g == 3),
                )
        uvW = workp.tile([NLOC, 2 * W], F32)
        nc.scalar.copy(uvW[:, 0:W], UVo[:])
        nc.scalar.copy(uvW[:, W:2 * W], UVt[:])
        uvv = workp.tile([NLOC, W], F32)
        nc.vector.tensor_mul(uvv[:], uvW[:, 0:W], uvW[:, W:2 * W])
        uvr = workp.tile([NLOC, 1], F32)
        nc.vector.reduce_sum(
            uvr[:], uvv.rearrange("p (x w) -> p x w", x=1), axis=AX.X
        )
        nc.tensor.matmul(
            ABp[0:1, 1:2], onesh[0:NLOC, :], uvr[:], skip_group_check=True
        )

        # ---- pack A/B partials, single tiny output DMA ----
        nc.scalar.copy(ABs[0:1, 1:4], ABp[0:1, 0:3])
        nc.sync.dma_start(ab_d, ABs[:])


def _build_nc():
    nc = bacc.Bacc("TRN2", target_bir_lowering=False, debug=False)
    o_d = nc.dram_tensor("o", [128, MAPS, W], F32R, kind="ExternalInput").ap()
    t_d = nc.dram_tensor("t", [128, MAPS, W], F32R, kind="ExternalInput").ap()
    zoz_d = nc.dram_tensor("zoz", [128, 2 * ROWS - 1], F32R, kind="ExternalInput").ap()
    mask_d = nc.dram_tensor("mask", [ROWS, 32], F32, kind="ExternalInput").ap()
    ab_d = nc.dram_tensor("ab", [1, 4], F32, kind="ExternalOutput").ap()
    with tile.TileContext(nc) as tc:
        _body(tc, o_d, t_d, zoz_d, mask_d, ab_d)
    nc.compile()
    return nc


_NC = None


def _get_nc():
    global _NC
    if _NC is None:
        _NC = _build_nc()
    return _NC


_ZOZ = np.zeros((128, 2 * ROWS - 1), np.float32)
_ZOZ[:, ROWS - 1] = 1.0
_MASK = np.zeros((ROWS, 32), np.float32)
for _r in range(ROWS):
    for _g in range(4):
        _MASK[_r, NLOC * _g + (4 * _r + _g) // C] = 1.0


def _make_in_maps(output, target):
    in_maps = []
    for i in range(NCORES):
        o = np.ascontiguousarray(
            output[i * NLOC:(i + 1) * NLOC].reshape(MAPS, H, W).transpose(1, 0, 2)
        )
        t = np.ascontiguousarray(
            target[i * NLOC:(i + 1) * NLOC].reshape(MAPS, H, W).transpose(1, 0, 2)
        )
        in_maps.append({"o": o, "t": t, "zoz": _ZOZ, "mask": _MASK})
    return in_maps


def _finish(results):
    A = 0.0
    B = 0.0
    for r in results:
        ab = r["ab"].astype(np.float64).ravel()
        A += ab[0] + ab[1]          # matched-cosine sums (h + w)
        B += ab[2] + ab[3]          # sum_n U_n.V_n (h + w)
    # sim_pos = 0.5*A/(N*C); sim = 0.5*B/N; loss = -log(sim_pos/sim)/(C*N)
    loss = -np.log(A / (C * B)) / (C * N)
    return np.float32(loss)


def kernel(output, target):
    output = np.asarray(output, dtype=np.float32)
    target = np.asarray(target, dtype=np.float32)
    nc = _get_nc()
    res = run_bass_kernel_spmd(nc, _make_in_maps(output, target), list(range(NCORES)))
    return _finish(res.results)


def profile(output, target):
    """Run once with NTFF tracing; returns max per-core HW exec time in ns."""
    output = np.asarray(output, dtype=np.float32)
    target = np.asarray(target, dtype=np.float32)
    nc = _get_nc()
    tmpdir = "/tmp/bass_prof_latest"
    shutil.rmtree(tmpdir, ignore_errors=True)
    os.makedirs(tmpdir, exist_ok=True)
    res = run_bass_kernel_spmd(
        nc, _make_in_maps(output, target), list(range(NCORES)), trace=True,
        tmpdir=tmpdir,
    )
    return res.exec_time_ns
